# revision 18
# baseline (speedup 1.0000x reference)
"""Bidirectional multi-head attention on 8 Trainium2 NeuronCores.

Problem: x:(2,2048,1024) f32, 16 heads of 64; qkv proj -> attention with
key-padding mask -> softmax -> out proj.  Sharding: batch (2) x head-groups
(4 groups of 4 heads) = 8 cores.  Each core computes its 4 heads' attention
context and a partial output projection (over its 256 context channels);
the host sums the 4 partial projections per batch (pure unshard + add).

On-chip layout is fully "transposed" (features on partitions, sequence on
the free axis), which makes every matmul contraction land on partitions
without any on-chip transposes:
  Q^T,K^T = W x^T          (lhsT = W^T tiles, rhs = x^T)
  V       = x W^T          (lhsT = x^T tiles, rhs = Wv^T)   [normal orient]
  S^T     = K^T' Q^T       (per 128-key tile; two heads row-tiled per pass)
  P^T     = exp(S^T/8 + maskbias[k])   [mask folded into per-partition bias]
  O_aug^T = V_aug^T P^T    (V_aug = [V | 1]; row 64 = softmax denominator)
  out^T  += Wp^T ctx^T     (partial over this core's 256 channels)

Softmax skips the running-max (scores are bounded: |s/8| < 4 for this
problem's scale) and folds the key mask into the exp bias (-30 => exp~0).
The denominator arrives for free as V_aug's ones-column, and 1/den is
partition-broadcast via a tiny SBUF->SBUF DMA.
"""

import ml_dtypes
import numpy as np

import bass_rust
import concourse.bass as bass
import concourse.mybir as mybir
import concourse.tile as tile
from concourse.bass_utils import run_bass_kernel_spmd
from concourse.vector_clock import ScopedClock

F32 = mybir.dt.float32
F32R = mybir.dt.float32r
BF16 = mybir.dt.bfloat16
BF16NP = ml_dtypes.bfloat16
AF = mybir.ActivationFunctionType

B, L, D, H, HD = 2, 2048, 1024, 16, 64
GROUPS = 4            # head groups per batch (one per core)
HPG = H // GROUPS     # 4 heads per group
CH = HPG * HD         # 256 context channels per group
NQB = L // 512        # q blocks of 512
NC_ = D // 128        # contraction chunks of 128 over the model dim
SCALE = 1.0 / float(np.sqrt(HD))


def _nkc_for(mask):
    """Key tiles (of 128) needed after host-side key compaction.

    Tokens are reordered per batch so unmasked keys come first; K/V/S/exp
    only cover the first nkc*128 token slots (padding gets exp-bias -30,
    i.e. weight ~e-30: numerically identical to full masking)."""
    mask = np.asarray(mask, bool)
    lc = max(int((~mask[b]).sum()) for b in range(mask.shape[0]))
    return max(1, -(-lc // 128))

MAXW = 1  # this walrus build accepts only ONE embedded sync wait per inst


class PatchedTileContext(tile.TileContext):
    """TileContext for walrus builds limited to one sync wait per
    instruction: excess waits move onto same-engine carrier NoOps committed
    immediately before the owning instruction (engines execute in order, so
    the wait set is honored at the same program point)."""

    def _split_waits(self, inst):
        si = inst.sync_info
        if si is None:
            return None
        waits = list(si.on_wait)
        if len(waits) <= MAXW:
            return None
        inst.sync_info = bass_rust.SyncInfo(
            on_wait=waits[-MAXW:], on_update=list(si.on_update)
        )
        carriers = []
        for i in range(0, len(waits) - MAXW, MAXW):
            nop = mybir.InstNoOp(
                name=self.nc.get_next_instruction_name(),
                engine=inst.engine,
                bass_nofuse=True,
            )
            nop.sync_info = bass_rust.SyncInfo(on_wait=waits[i : i + MAXW], on_update=[])
            carriers.append(nop)
        return carriers

    def _commit_instruction(self, inst, lazy_reg_writes: bool = True):
        carriers = self._split_waits(inst)
        if carriers:
            for nop in carriers:
                super()._commit_instruction(nop)
        return super()._commit_instruction(inst, lazy_reg_writes)

    def _drain_and_barrier(self, tick_clock, wait_clock):
        drain_inst = self.nc.sync.drain()
        wait_clock.add_sem_waits(
            drain_inst.ins, ScopedClock({None: tick_clock.global_clock})
        )
        waits = list(drain_inst.ins.sync_info.on_wait)
        if len(waits) > MAXW:
            drain_inst.ins.sync_info = bass_rust.SyncInfo(
                on_wait=waits[:MAXW], on_update=[]
            )
            for i in range(MAXW, len(waits), MAXW):
                extra = self.nc.sync.drain()
                extra.ins.sync_info = bass_rust.SyncInfo(
                    on_wait=waits[i : i + MAXW], on_update=[]
                )
        self.nc.all_engine_barrier()
        assert self.sems is not None
        popped = self.nc._tile_sem_poison_stack.pop()
        assert popped is self._sem_poison
        self.nc.clear_and_free_semaphores(list(self.sems.allocated().values()))
        self.nc.all_engine_barrier()


def _build_nc(niter=1, nkc=L // 128):
    """niter > 1 replays the whole kernel body N times inside one NEFF —
    used only for timing (amortizes the large fixed per-dispatch overhead
    of this container's axon/PJRT path)."""
    nc = bass.Bass()
    xT_h = nc.dram_tensor("xT", [D, L], BF16, kind="ExternalInput")
    wqkT_h = nc.dram_tensor("wqkT", [D, 2 * CH], BF16, kind="ExternalInput")
    wvT_h = nc.dram_tensor("wvT", [D, CH], BF16, kind="ExternalInput")
    wpT_h = nc.dram_tensor("wpT", [CH, D], BF16, kind="ExternalInput")
    bqk_h = nc.dram_tensor("bqk", [128, 4], F32, kind="ExternalInput")
    bvb_h = nc.dram_tensor("bvb", [128, CH], F32, kind="ExternalInput")
    bp_h = nc.dram_tensor("bp", [128, 8], F32, kind="ExternalInput")
    mb_h = nc.dram_tensor("mb", [128, nkc], F32, kind="ExternalInput")
    outT_h = nc.dram_tensor("outT", [D, L], BF16, kind="ExternalOutput")

    with PatchedTileContext(nc) as tc:
        for it in range(niter):
            _emit_iteration(
                tc, it, nkc, xT_h, wqkT_h, wvT_h, wpT_h, bqk_h, bvb_h, bp_h,
                mb_h, outT_h,
            )
    return nc


def _emit_iteration(
    tc, it, nkc, xT_h, wqkT_h, wvT_h, wpT_h, bqk_h, bvb_h, bp_h, mb_h, outT_h
):
    nc = tc.nc
    nkb = -(-nkc // 4)  # 512-wide key blocks for the K projection
    with (
        tc.tile_pool(name=f"consts{it}", bufs=1) as consts,
        tc.tile_pool(name=f"persist{it}", bufs=1) as persist,
    ):
        # small constants
        bqk_sb = consts.tile([128, 4], F32)
        nc.sync.dma_start(bqk_sb[:], bqk_h[:])
        bvb_sb = consts.tile([128, HPG, HD], F32)
        nc.sync.dma_start(bvb_sb[:], bvb_h[:].rearrange("p (h d) -> p h d", h=HPG))
        bp_sb = consts.tile([128, 8], F32)
        nc.sync.dma_start(bp_sb[:], bp_h[:])
        mb_sb = consts.tile([128, nkc], F32)
        nc.sync.dma_start(mb_sb[:], mb_h[:])
        ones_sb = consts.tile([1, HD], F32R)
        nc.vector.memset(ones_sb[:].bitcast(F32), 1.0)
        wp_sb = consts.tile([128, 2, D], BF16)
        nc.scalar.dma_start(
            wp_sb[:], wpT_h[:].rearrange("(c p) m -> p c m", p=128)
        )

        # persistent activations
        QT_sb = persist.tile([128, 2, L], BF16)   # [2-head lanes, hp, q]
        KT_sb = persist.tile([128, 2, nkb * 512], BF16)
        Vaug_sb = persist.tile([128, nkc, HPG, HD + 1], BF16)
        nc.vector.memset(Vaug_sb[:, :, :, HD : HD + 1], 1.0)

        # ------------- unified pipeline ------------------------------
        # Unit u = (qb, hp): 8 units, hp0 first.  S+exp for a unit runs two
        # units ahead of its O-accumulation (P buffered in bf16), so the
        # scalar engine starts exp'ing ~15us in (right after the Q/K-hp0
        # wave) and never starves.  V and QK-hp1 projection waves are
        # interleaved with the run-ahead S/exp stream.
        units = [(qb, hp) for hp in (0, 1) for qb in range(NQB)]
        with tc.tile_pool(name=f"xw{it}", bufs=1) as xw:
            wqk_sb = xw.tile([128, NC_, 2 * CH], BF16)
            wqk_r = wqkT_h[:].rearrange("(c p) m -> p c m", p=128)
            # hp0's weight columns (m-tiles 0 and 2) first — they gate wave 0
            nc.sync.dma_start(wqk_sb[:, :, 0:128], wqk_r[:, :, 0:128])
            nc.sync.dma_start(wqk_sb[:, :, 256:384], wqk_r[:, :, 256:384])
            wv_sb = xw.tile([128, NC_, CH], BF16)
            xT_sb = xw.tile([128, NC_, L], BF16)
            xT_r = xT_h[:].rearrange("(c p) l -> p c l", p=128)
            # split input streams across both HWDGE rings (SP + ACT)
            for c in range(NC_):
                eng = nc.scalar if c % 2 == 0 else nc.sync
                eng.dma_start(xT_sb[:, c, :], xT_r[:, c, :])
            nc.scalar.dma_start(wqk_sb[:, :, 128:256], wqk_r[:, :, 128:256])
            nc.scalar.dma_start(wqk_sb[:, :, 384:512], wqk_r[:, :, 384:512])
            nc.scalar.dma_start(
                wv_sb[:], wvT_h[:].rearrange("(c p) m -> p c m", p=128)
            )

            def wave_mm(pool_, wave, interleave=None):
                tiles = []
                for kind, a, b in wave:
                    width = 512 if kind == "qk" else CH
                    tiles.append(
                        pool_.tile([128, width], F32, tag="a",
                                   name=f"aw{len(tiles)}")
                    )
                for c in range(NC_):
                    for (kind, a, b), ps in zip(wave, tiles):
                        if kind == "qk":
                            nc.tensor.matmul(
                                ps[:],
                                wqk_sb[:, c, a * 128 : (a + 1) * 128],
                                xT_sb[:, c, b * 512 : (b + 1) * 512],
                                start=(c == 0),
                                stop=(c == NC_ - 1),
                                skip_group_check=True,
                            )
                        else:
                            nc.tensor.matmul(
                                ps[:],
                                xT_sb[:, c, a * 128 : (a + 1) * 128],
                                wv_sb[:, c, :],
                                start=(c == 0),
                                stop=(c == NC_ - 1),
                                skip_group_check=True,
                            )
                    if interleave is not None:
                        interleave()
                for (kind, a, b), ps in zip(wave, tiles):
                    if kind == "qk":
                        dst = QT_sb if a < 2 else KT_sb
                        nc.vector.tensor_scalar_add(
                            out=dst[:, a % 2, b * 512 : (b + 1) * 512],
                            in0=ps[:],
                            scalar1=bqk_sb[:, a : a + 1],
                        )
                    else:
                        nc.vector.tensor_add(
                            out=Vaug_sb[:, a, :, 0:HD],
                            in0=ps[:].rearrange("p (h d) -> p h d", h=HPG),
                            in1=bvb_sb[:],
                        )

            # wave 0: everything attention-hp0 needs (Q-hp0 all blocks,
            # K-hp0 all key blocks) — 4 + nkb <= 8 PSUM banks
            with tc.tile_pool(name=f"w0{it}", bufs=8, space="PSUM") as ps8:
                wave_mm(ps8, [("qk", 0, lb) for lb in range(NQB)]
                        + [("qk", 2, lb) for lb in range(nkb)])

            with (
                tc.tile_pool(name=f"psS{it}", bufs=2, space="PSUM") as psS,
                tc.tile_pool(name=f"p_sb{it}", bufs=2 * nkc + 2) as p_pool,
                tc.tile_pool(name=f"norm{it}", bufs=4) as norm_pool,
                tc.tile_pool(name=f"ctx{it}", bufs=1) as ctx_pool,
                tc.tile_pool(name=f"stage{it}", bufs=4) as stage,
            ):
                P = {}

                def s_exp(i, kt):
                    qb, hp = units[i]
                    qsl = slice(qb * 512, (qb + 1) * 512)
                    ksl = slice(kt * 128, (kt + 1) * 128)
                    s_ps = psS.tile([128, 2, 512], F32, tag="sh")
                    nc.tensor.matmul(
                        s_ps[:, 0, :],
                        KT_sb[0:64, hp, ksl],
                        QT_sb[0:64, hp, qsl],
                        start=True,
                        stop=True,
                        skip_group_check=True,
                    )
                    nc.tensor.matmul(
                        s_ps[:, 1, :],
                        KT_sb[64:128, hp, ksl],
                        QT_sb[64:128, hp, qsl],
                        start=True,
                        stop=True,
                        tile_position=(64, 0),
                        skip_group_check=True,
                    )
                    p_sb = p_pool.tile([128, 2, 512], BF16, tag="p")
                    nc.scalar.activation(
                        out=p_sb[:],
                        in_=s_ps[:],
                        func=AF.Exp,
                        bias=mb_sb[:, kt : kt + 1],
                        scale=float(SCALE),
                    )
                    P[(i, kt)] = p_sb

                # run-ahead S/exp for units 0-1, spread between the V and
                # QK-hp1 wave chunks so the PE never idles on score PSUM
                todo = [(i, kt) for i in (0, 1) for kt in range(nkc)]
                ti = iter(todo)

                def il():
                    nxt = next(ti, None)
                    if nxt is not None:
                        s_exp(*nxt)

                vts = [("v", lt, 0) for lt in range(nkc)]
                with tc.tile_pool(name=f"wv{it}", bufs=4, space="PSUM") as ps4:
                    for i0 in range(0, nkc, 4):
                        wave_mm(ps4, vts[i0 : i0 + 4], interleave=il)
                    wave_mm(ps4, [("qk", 1, lb) for lb in range(NQB)],
                            interleave=il)
                    wave_mm(ps4, [("qk", 3, lb) for lb in range(nkb)],
                            interleave=il)
                    for rem in ti:
                        s_exp(*rem)

                with (
                    tc.tile_pool(name=f"psO{it}", bufs=2, space="PSUM") as psO,
                    tc.tile_pool(name=f"psT{it}", bufs=1, space="PSUM") as psT,
                ):
                    def emit_proj(qb, ctx_sb):
                        qsl = slice(qb * 512, (qb + 1) * 512)
                        for mt in range(8):
                            msl = slice(mt * 128, (mt + 1) * 128)
                            pr = psT.tile([128, 512], F32, tag="pr")
                            for hp in range(2):
                                nc.tensor.matmul(
                                    pr[:],
                                    wp_sb[:, hp, msl],
                                    ctx_sb[:, hp, :],
                                    start=(hp == 0),
                                    stop=(hp == 1),
                                    skip_group_check=True,
                                )
                            st = stage.tile([128, 512], BF16, tag="st")
                            nc.vector.tensor_scalar_add(
                                out=st[:], in0=pr[:],
                                scalar1=bp_sb[:, mt : mt + 1],
                            )
                            nc.sync.dma_start(outT_h[msl, qsl], st[:])

                    ctxs = [
                        ctx_pool.tile([128, 2, 512], BF16, tag=f"c{qb}",
                                      name=f"ctx{qb}")
                        for qb in range(NQB)
                    ]
                    pending = None
                    for i, (qb, hp) in enumerate(units):
                        o_ps = [
                            psO.tile([HD + 1, 512], F32, tag="o", name=f"o{j}")
                            for j in range(2)
                        ]
                        for kt in range(nkc):
                            p_sb = P.pop((i, kt))
                            for hh in range(2):
                                nc.tensor.matmul(
                                    o_ps[hh][:],
                                    Vaug_sb[:, kt, 2 * hp + hh, :],
                                    p_sb[:, hh, :],
                                    start=(kt == 0),
                                    stop=(kt == nkc - 1),
                                    skip_group_check=True,
                                )
                            if i + 2 < len(units):
                                s_exp(i + 2, kt)
                            if kt == 2 and pending is not None:
                                emit_proj(*pending)
                                pending = None
                        # drain O to SBUF (frees PSUM for the next unit)
                        o_sb = norm_pool.tile([HD + 1, 2, 512], F32, tag="osb")
                        for hh in range(2):
                            nc.vector.tensor_copy(o_sb[:, hh, :], o_ps[hh][:])
                        # normalize: ctx = O[0:64] * broadcast(1/den)
                        r_sb = norm_pool.tile([1, 2, 512], F32R, tag="r")
                        with nc.allow_low_precision(
                            reason="1/denominator consumed as f32r"
                        ):
                            nc.vector.reciprocal(
                                out=r_sb[:], in_=o_sb[HD : HD + 1, :, :]
                            )
                        for hh in range(2):
                            bc_ps = psT.tile([HD, 512], F32, tag="bc")
                            nc.tensor.matmul(
                                bc_ps[:],
                                ones_sb[:],
                                r_sb[:, hh, :],
                                start=True,
                                stop=True,
                                skip_group_check=True,
                            )
                            nc.vector.tensor_mul(
                                out=ctxs[qb][hh * 64 : (hh + 1) * 64, hp, :],
                                in0=o_sb[0:HD, hh, :],
                                in1=bc_ps[:],
                            )
                        if hp == 1:
                            pending = (qb, ctxs[qb])
                    emit_proj(*pending)


_NC_CACHE = {}


def _get_nc(nkc=L // 128):
    if nkc not in _NC_CACHE:
        _NC_CACHE[nkc] = _build_nc(nkc=nkc)
    return _NC_CACHE[nkc]


def _perm_for(mask, b):
    """Token order with unmasked (key-visible) tokens first. Queries are
    order-independent here (no positional op inside the module), so the
    host just un-permutes the output columns."""
    return np.argsort(np.asarray(mask, bool)[b], kind="stable")


def _prep_core_inputs(core, x, mask, wqkv, bqkv, wproj, bproj):
    b, g = core // GROUPS, core % GROUPS
    nkc = _nkc_for(mask)
    perm = _perm_for(mask, b)
    lc = int((~np.asarray(mask, bool)[b]).sum())
    sl = slice(g * CH, (g + 1) * CH)
    wq = wqkv[0 * D + g * CH : 0 * D + (g + 1) * CH]
    wk = wqkv[1 * D + g * CH : 1 * D + (g + 1) * CH]
    wv = wqkv[2 * D + g * CH : 2 * D + (g + 1) * CH]
    bq = bqkv[0 * D + g * CH : 0 * D + (g + 1) * CH]
    bk = bqkv[1 * D + g * CH : 1 * D + (g + 1) * CH]
    bv = bqkv[2 * D + g * CH : 2 * D + (g + 1) * CH]
    bpc = bproj if g == 0 else np.zeros_like(bproj)
    mb = np.where(np.arange(nkc * 128) < lc, np.float32(0.0),
                  np.float32(-30.0))
    return {
        "xT": np.ascontiguousarray(x[b].T.astype(BF16NP)[:, perm]),
        "wqkT": np.ascontiguousarray(
            np.concatenate([wq, wk], axis=0).T.astype(BF16NP)
        ),
        "wvT": np.ascontiguousarray(wv.T.astype(BF16NP)),
        "wpT": np.ascontiguousarray(wproj[:, sl].T.astype(BF16NP)),
        "bqk": np.ascontiguousarray(
            np.concatenate([bq, bk]).reshape(4, 128).T
        ),
        "bvb": np.ascontiguousarray(np.broadcast_to(bv, (128, CH))),
        "bp": np.ascontiguousarray(bpc.reshape(8, 128).T),
        "mb": np.ascontiguousarray(mb.reshape(nkc, 128).T),
    }


def kernel(x, mask, wqkv, bqkv, wproj, bproj, _trace=False, _trace_kwargs=None):
    x = np.asarray(x, np.float32)
    mask = np.asarray(mask, bool)
    wqkv = np.asarray(wqkv, np.float32)
    bqkv = np.asarray(bqkv, np.float32)
    wproj = np.asarray(wproj, np.float32)
    bproj = np.asarray(bproj, np.float32)

    nc = _get_nc(_nkc_for(mask))
    in_maps = [
        _prep_core_inputs(c, x, mask, wqkv, bqkv, wproj, bproj) for c in range(8)
    ]
    kw = {}
    if _trace:
        kw = {"trace": True, **(_trace_kwargs or {})}
    res = run_bass_kernel_spmd(nc, in_maps, list(range(8)), **kw)
    out = np.empty((B, L, D), np.float32)
    for b in range(B):
        acc = np.asarray(res.results[b * GROUPS + 0]["outT"]).astype(np.float32)
        for g in range(1, GROUPS):
            acc += np.asarray(res.results[b * GROUPS + g]["outT"]).astype(
                np.float32
            )
        out[b][_perm_for(mask, b)] = acc.T
    if _trace:
        return out, res
    return out

